# revision 15
# baseline (speedup 1.0000x reference)
"""AdSBHNet trapezoid-integral kernel for 8 TRN2 NeuronCores.

Math (all-real reformulation of the complex reference):
  poly(c,z) = sum_{i=1..5} c_i z^i ;  f = (1-z^4) e^{poly(a,z)} ; g = e^{poly(b,z)}/(1-z^4)
  z = zs*u on a uniform u-grid (Nu_L=2000 / Nu_V=1500), du == h everywhere.
  L: w  = A/(D+eps+i*eps) - 1 + eps(1+i),  A = zs^4 f(z), D = z^4 f(zs)
     integrand = sqrt(g)/sqrt(w);  L = (2/pi) * zs*h * sum(w_j * integrand_j)
  V: inner = 1 - Y/(X+eps+i*eps) + eps(1+i), Y = z^4 f(zs), X = zs^4 f(z)
     term = sqrt(f g)/sqrt(inner) - 1; integrand = term/(z^2+eps(1+i))
     V = 2pi*zs*h*sum(w_j integrand_j) - 2pi/zs
  Complex sqrt of w=re+i*im with r=|w|: sqrt(w) = p + i*q, p=sqrt((r+re)/2),
  q=sign(im)*sqrt((r-re)/2); 1/sqrt(w) = (p-i*q)/r.  For sqrt(g)/sqrt(w):
    sqrt(g)*p/r = sqrt(G*(r+re)), sqrt(g)*|q|/r = sqrt(G*(r-re)),
    G = g/(2 r^2) > 0.
  Numerical stability: r-|re| catastrophically cancels, so compute
    rlarge = r+|re| (well conditioned) and rsmall = im^2/rlarge
  (identity r^2-re^2 = im^2) and route sqrt(G*rlarge)/sqrt(G*rsmall) to the
  p/q slots by sign(re) with predicated copies — same branch structure as
  the reference's complex sqrt.

Polynomial evaluation over the [128,N] grid is a K=6 TensorE matmul:
  poly(c, zs_r*u_j) = sum_i (zs_r^i) * (c_i u_j^i);  lhsT = zs-powers [6,128],
  rhs = c-scaled u-powers [6,N]; row 0 (k=0) carries trapezoid ln-weights.

Sharding: pure data parallel, zs batch split 8 ways; a/b replicated.
"""

import math
import sys

import numpy as np

sys.path.insert(0, "/opt/trn_rl_repo")

import concourse.bass as bass
import concourse.bacc as bacc
import concourse.mybir as mybir
from concourse import bass_utils
from concourse.tile import TileContext

F32 = mybir.dt.float32
I32 = mybir.dt.int32
OP = mybir.AluOpType
AF = mybir.ActivationFunctionType

EPS = 1e-6
EPS2 = EPS * EPS
NU_L = 2000
NU_V = 1500
B = 8192
NCORES = 8
BLOC = B // NCORES       # 1024 rows per core
NT = BLOC // 128         # 8 row-tiles per core
H_L = (1.0 - 2 * EPS) / (NU_L - 1)
H_V = (1.0 - 2 * EPS) / (NU_V - 1)
LN2 = math.log(2.0)
LNW2 = math.log(0.25)    # ln(w^2) at trapezoid endpoints (w=0.5)
MMC = 512                # matmul free-dim chunk


def _chunks(n):
    return [(c, min(c + MMC, n)) for c in range(0, n, MMC)]


def build_nc():
    nc = bacc.Bacc("TRN2", target_bir_lowering=False, debug=False, num_devices=NCORES)
    a_d = nc.declare_dram_parameter("a", [5], F32, isOutput=False)
    b_d = nc.declare_dram_parameter("b", [5], F32, isOutput=False)
    zs_d = nc.declare_dram_parameter("zs", [BLOC], F32, isOutput=False)
    out_d = nc.declare_dram_parameter("out", [4, BLOC], F32, isOutput=True)

    with TileContext(nc) as tc:
        with (
            tc.tile_pool(name="cst", bufs=1) as cst,
            tc.tile_pool(name="wk", bufs=1) as wk,
            tc.tile_pool(name="ps", bufs=1, space="PSUM") as pspool,
        ):
            v = nc.vector
            sc = nc.scalar

            def W(tag, n=NU_L, dt=F32, nm=None):
                return wk.tile([128, n], dt, tag=tag, name=nm or f"t{tag}")

            # ---------------- setup: per-row quantities ----------------
            zcol = cst.tile([128, NT], F32)          # zs, col t = tile t
            nc.sync.dma_start(out=zcol[:], in_=zs_d[:].rearrange("(t p) -> p t", p=128))
            zrow = cst.tile([1, BLOC], F32)
            nc.sync.dma_start(out=zrow[:], in_=zs_d[:].rearrange("(o n) -> o n", o=1))

            aext = cst.tile([6, 1], F32)
            bext = cst.tile([6, 1], F32)
            v.memset(aext[:], 0.0)
            v.memset(bext[:], 0.0)
            nc.sync.dma_start(out=aext[1:6, 0:1], in_=a_d[:])
            nc.sync.dma_start(out=bext[1:6, 0:1], in_=b_d[:])
            abext = cst.tile([6, 1], F32)
            v.tensor_tensor(abext[:], aext[:], bext[:], OP.add)

            ones6 = cst.tile([1, 6], F32)
            v.memset(ones6[:], 1.0)

            # kcol6 = [0,1,2,3,4,5]; row 0 is the const-1 / weight row
            kcol_i = cst.tile([6, 1], I32)
            nc.gpsimd.iota(kcol_i[:], pattern=[[1, 1]], base=0, channel_multiplier=1)
            kcol6 = cst.tile([6, 1], F32)
            v.tensor_copy(kcol6[:], kcol_i[:])

            # ZPow6 [6, BLOC]: row k = zs^k (row 0 = 1) via exp(k ln zs)
            lnz = cst.tile([1, BLOC], F32)
            sc.activation(lnz[:], zrow[:], AF.Ln)
            ps6 = pspool.tile([6, BLOC], F32, tag="pa", name="ps6")
            for c0, c1 in _chunks(BLOC):
                nc.tensor.matmul(ps6[:, c0:c1], ones6[:], lnz[:, c0:c1], start=True, stop=True)
            klnz = cst.tile([6, BLOC], F32)
            v.tensor_scalar(klnz[:], ps6[:], kcol6[:], None, OP.mult)
            zpow = cst.tile([6, BLOC], F32)
            sc.activation(zpow[:], klnz[:], AF.Exp)

            # per-row [128, NT] tiles
            zs2c = cst.tile([128, NT], F32)
            v.tensor_tensor(zs2c[:], zcol[:], zcol[:], OP.mult)
            zs4c = cst.tile([128, NT], F32)
            v.tensor_tensor(zs4c[:], zs2c[:], zs2c[:], OP.mult)
            nzs4c = cst.tile([128, NT], F32)
            v.tensor_scalar(nzs4c[:], zs4c[:], -1.0, None, OP.mult)
            lnzs4 = cst.tile([128, NT], F32)
            sc.activation(lnzs4[:], zs4c[:], AF.Ln)

            # pa(zs) for all rows via 8 tiny matmuls -> [128, NT]
            ps_pz = pspool.tile([128, NT], F32, tag="pb", name="ps_pz")
            for t in range(NT):
                nc.tensor.matmul(
                    ps_pz[:, t : t + 1], zpow[:, t * 128 : (t + 1) * 128], aext[:],
                    start=True, stop=True,
                )
            e_paz = cst.tile([128, NT], F32)
            sc.activation(e_paz[:], ps_pz[:], AF.Exp)
            omzs4 = cst.tile([128, NT], F32)
            v.tensor_scalar(omzs4[:], zs4c[:], -1.0, 1.0, OP.mult, OP.add)
            fzs = cst.tile([128, NT], F32)
            v.tensor_tensor(fzs[:], e_paz[:], omzs4[:], OP.mult)
            c1c = cst.tile([128, NT], F32)
            v.tensor_tensor(c1c[:], zs4c[:], fzs[:], OP.mult)

            # scales
            sL = cst.tile([128, NT], F32)
            v.tensor_scalar(sL[:], zcol[:], 2.0 * H_L / math.pi, None, OP.mult)
            sLn = cst.tile([128, NT], F32)
            v.tensor_scalar(sLn[:], zcol[:], -2.0 * H_L / math.pi, None, OP.mult)
            sV = cst.tile([128, NT], F32)
            v.tensor_scalar(sV[:], zcol[:], 2.0 * math.pi * H_V, None, OP.mult)
            sVn = cst.tile([128, NT], F32)
            v.tensor_scalar(sVn[:], zcol[:], -2.0 * math.pi * H_V, None, OP.mult)
            invz = cst.tile([128, NT], F32)
            invz_s = cst.tile([128, NT], F32)
            v.reciprocal_approx_accurate(invz[:], zcol[:], invz_s[:])

            # ---------------- setup: u-grids ----------------
            io_c = W("w0", dt=I32, nm="io_c")
            nc.gpsimd.iota(io_c[:], pattern=[[1, NU_L]], base=0, channel_multiplier=0)
            iof = W("w1", nm="iof")
            v.tensor_copy(iof[:], io_c[:])
            io6_c = wk.tile([6, NU_L], I32, tag="w2", name="io6_c")
            nc.gpsimd.iota(io6_c[:], pattern=[[1, NU_L]], base=0, channel_multiplier=0)
            iof6 = wk.tile([6, NU_L], F32, tag="w3", name="iof6")
            v.tensor_copy(iof6[:], io6_c[:])

            grids = {}
            for gname, N, H in (("L", NU_L, H_L), ("V", NU_V, H_V)):
                u1 = W("w4", N, nm=f"u1{gname}")
                v.tensor_scalar(u1[:], iof[:, 0:N], H, EPS, OP.mult, OP.add)
                if gname == "V":
                    u2 = cst.tile([128, N], F32, name="u2V")
                else:
                    u2 = W("w5", N, nm="u2L")
                v.tensor_tensor(u2[:], u1[:], u1[:], OP.mult)
                u4 = cst.tile([128, N], F32, name=f"u4{gname}")
                v.tensor_tensor(u4[:], u2[:], u2[:], OP.mult)

                # Upow6 [6,N] = u^k rows (row0 = 1) via exp(k ln u)
                u16 = wk.tile([6, N], F32, tag="w6", name=f"u16{gname}")
                v.tensor_scalar(u16[:], iof6[:, 0:N], H, EPS, OP.mult, OP.add)
                lnu = wk.tile([6, N], F32, tag="w7", name=f"lnu{gname}")
                sc.activation(lnu[:], u16[:], AF.Ln)
                klnu = wk.tile([6, N], F32, tag="w8", name=f"klnu{gname}")
                v.tensor_scalar(klnu[:], lnu[:], kcol6[:], None, OP.mult)
                upow = wk.tile([6, N], F32, tag="w9", name=f"upow{gname}")
                sc.activation(upow[:], klnu[:], AF.Exp)

                ra = cst.tile([6, N], F32, name=f"ra{gname}")
                v.tensor_scalar(ra[:], upow[:], aext[:], None, OP.mult)
                if gname == "L":
                    rb = cst.tile([6, N], F32, name="rbL")
                    v.tensor_scalar(rb[:], upow[:], bext[:], None, OP.mult)
                    # trapezoid endpoint ln-weights in row 0 of rb:
                    # iota = p + j (resp. p + N-1-j) is 0 only at the target elem
                    nc.gpsimd.affine_select(
                        out=rb[:], in_=rb[:], pattern=[[1, N]],
                        compare_op=OP.is_gt, fill=LNW2, base=0,
                        channel_multiplier=1,
                    )
                    nc.gpsimd.affine_select(
                        out=rb[:], in_=rb[:], pattern=[[-1, N]],
                        compare_op=OP.is_gt, fill=LNW2, base=N - 1,
                        channel_multiplier=1,
                    )
                    grids["L"] = (u4, ra, rb)
                else:
                    rab = cst.tile([6, N], F32, name="rabV")
                    v.tensor_scalar(rab[:], upow[:], abext[:], None, OP.mult)
                    grids["V"] = (u2, u4, ra, rab)

            # accumulators & scratch
            accLre = cst.tile([128, NT], F32)
            accLim = cst.tile([128, NT], F32)
            accVre = cst.tile([128, NT], F32)
            accVim = cst.tile([128, NT], F32)
            dum = cst.tile([128, NU_L], F32)   # ACT accum scratch, never read
            nln2 = cst.tile([128, 1], F32)     # -ln2 bias column
            v.memset(nln2[:], -LN2)

            # ---------------- main loop ----------------
            U4L, RA_L, RB_L = grids["L"]
            U2V, U4V, RA_V, RAB_V = grids["V"]

            for t in range(NT):
                lhs = zpow[:, t * 128 : (t + 1) * 128]
                nzs4_t = nzs4c[:, t : t + 1]
                c1_t = c1c[:, t : t + 1]
                ln4_t = lnzs4[:, t : t + 1]

                # ======== L integral (N=2000) ========
                N = NU_L
                pa_ps = pspool.tile([128, N], F32, tag="pa", name="paL")
                pb_ps = pspool.tile([128, N], F32, tag="pb", name="pbL")
                for c0, c1 in _chunks(N):
                    nc.tensor.matmul(pa_ps[:, c0:c1], lhs, RA_L[:, c0:c1], start=True, stop=True)
                for c0, c1 in _chunks(N):
                    nc.tensor.matmul(pb_ps[:, c0:c1], lhs, RB_L[:, c0:c1], start=True, stop=True)

                e_a2 = W("w0")
                sc.activation(e_a2[:], pa_ps[:], AF.Exp, bias=ln4_t, scale=1.0)
                e_b2 = W("w1")
                sc.activation(e_b2[:], pb_ps[:], AF.Exp, bias=nln2[:, 0:1], scale=1.0)

                omz4 = W("w2")
                v.tensor_scalar(omz4[:], U4L[:], nzs4_t, 1.0, OP.mult, OP.add)
                Dp = W("w3")
                v.tensor_scalar(Dp[:], U4L[:], c1_t, EPS, OP.mult, OP.add)
                X = W("w4")
                v.tensor_tensor(X[:], omz4[:], e_a2[:], OP.mult)
                sqDp = W("w5")
                sc.activation(sqDp[:], Dp[:], AF.Square)
                n2 = W("w6")
                v.tensor_scalar(n2[:], sqDp[:], EPS2, None, OP.add)
                rn2 = W("w5")
                v.reciprocal_approx_fast(rn2[:], n2[:])
                t_ = W("w6")
                v.tensor_tensor(t_[:], X[:], rn2[:], OP.mult)
                tDp = W("w4")
                v.tensor_tensor(tDp[:], t_[:], Dp[:], OP.mult)
                re = W("w5")
                v.tensor_scalar(re[:], tDp[:], -(1.0 - EPS), None, OP.add)
                im = W("w7")
                v.tensor_scalar(im[:], t_[:], -EPS, EPS, OP.mult, OP.add)
                sgn = W("w8")
                sc.activation(sgn[:], im[:], AF.Sign)
                sqre = W("w4")
                sc.activation(sqre[:], re[:], AF.Square)
                sqim = W("w9")
                sc.activation(sqim[:], im[:], AF.Square)
                r2s = W("w6")
                v.tensor_tensor(r2s[:], sqre[:], sqim[:], OP.add)
                r_ = W("w4")
                sc.activation(r_[:], r2s[:], AF.Sqrt)
                d2 = W("w7")
                v.tensor_tensor(d2[:], omz4[:], r2s[:], OP.mult)
                rd2 = W("w2")
                v.reciprocal_approx_fast(rd2[:], d2[:])
                G = W("w6")
                v.tensor_tensor(G[:], e_b2[:], rd2[:], OP.mult)
                absre = W("w2")
                sc.activation(absre[:], re[:], AF.Abs)
                rlg = W("w3")
                v.tensor_tensor(rlg[:], absre[:], r_[:], OP.add)
                glg = W("w7")
                v.tensor_tensor(glg[:], G[:], rlg[:], OP.mult)
                SS = W("w1")                     # -> becomes igq after swap
                sc.activation(SS[:], glg[:], AF.Sqrt)
                rcl = W("w4")
                v.reciprocal_approx_fast(rcl[:], rlg[:])
                t1 = W("w2")
                v.tensor_tensor(t1[:], sqim[:], rcl[:], OP.mult)
                gsm = W("w7")
                v.tensor_tensor(gsm[:], G[:], t1[:], OP.mult)
                TTs = W("w0")                    # -> becomes igre after swap
                sc.activation(TTs[:], gsm[:], AF.Sqrt)
                TTs2 = W("w3")
                sc.activation(TTs2[:], gsm[:], AF.Sqrt)
                m = W("w4")
                v.tensor_scalar(m[:], re[:], 0.0, None, OP.is_ge)
                # igre = m ? SS : TTs ; igq = m ? TTs : SS
                # (f32 mask 1.0/0.0 bitcast to i32 is nonzero/zero)
                v.copy_predicated(TTs[:], m[:].bitcast(I32), SS[:])
                v.copy_predicated(SS[:], m[:].bitcast(I32), TTs2[:])
                sc.activation(dum[:, 0:N], TTs[:], AF.Copy,
                              accum_out=accLre[:, t : t + 1])
                igqs = W("w2")
                v.scalar_tensor_tensor(
                    igqs[:], SS[:], 1.0, sgn[:], OP.mult, OP.mult,
                    accum_out=accLim[:, t : t + 1],
                )

                # ======== V integral (N=1500) ========
                N = NU_V
                pa_ps = pspool.tile([128, N], F32, tag="pa", name="paV")
                pab_ps = pspool.tile([128, N], F32, tag="pb", name="pabV")
                for c0, c1 in _chunks(N):
                    nc.tensor.matmul(pa_ps[:, c0:c1], lhs, RA_V[:, c0:c1], start=True, stop=True)
                for c0, c1 in _chunks(N):
                    nc.tensor.matmul(pab_ps[:, c0:c1], lhs, RAB_V[:, c0:c1], start=True, stop=True)

                e_a2 = W("w0", N)
                sc.activation(e_a2[:], pa_ps[:], AF.Exp, bias=ln4_t, scale=1.0)
                e_ab2 = W("w1", N)
                sc.activation(e_ab2[:], pab_ps[:], AF.Exp, bias=nln2[:, 0:1], scale=1.0)

                omz4 = W("w2", N)
                v.tensor_scalar(omz4[:], U4V[:], nzs4_t, 1.0, OP.mult, OP.add)
                Y = W("w3", N)
                v.tensor_scalar(Y[:], U4V[:], c1_t, None, OP.mult)
                X = W("w4", N)
                v.tensor_tensor(X[:], omz4[:], e_a2[:], OP.mult)
                Xp = W("w0", N)
                v.tensor_scalar(Xp[:], X[:], EPS, None, OP.add)
                sqXp = W("w2", N)
                sc.activation(sqXp[:], Xp[:], AF.Square)
                n2v = W("w4", N)
                v.tensor_scalar(n2v[:], sqXp[:], EPS2, None, OP.add)
                rn2v = W("w2", N)
                v.reciprocal_approx_fast(rn2v[:], n2v[:])
                t2 = W("w4", N)
                v.tensor_tensor(t2[:], Y[:], rn2v[:], OP.mult)
                t2Xp = W("w3", N)
                v.tensor_tensor(t2Xp[:], t2[:], Xp[:], OP.mult)
                re2 = W("w0", N)
                v.tensor_scalar(re2[:], t2Xp[:], -1.0, 1.0 + EPS, OP.mult, OP.add)
                im2 = W("w2", N)
                v.tensor_scalar(im2[:], t2[:], EPS, EPS, OP.mult, OP.add)
                sqre2 = W("w3", N)
                sc.activation(sqre2[:], re2[:], AF.Square)
                sqim2 = W("w4", N)
                sc.activation(sqim2[:], im2[:], AF.Square)
                r2s2 = W("w2", N)
                v.tensor_tensor(r2s2[:], sqre2[:], sqim2[:], OP.add)
                r2v = W("w3", N)
                sc.activation(r2v[:], r2s2[:], AF.Sqrt)
                rr2 = W("w5", N)
                v.reciprocal_approx_fast(rr2[:], r2s2[:])
                G2 = W("w2", N)
                v.tensor_tensor(G2[:], e_ab2[:], rr2[:], OP.mult)
                absre2 = W("w6", N)
                sc.activation(absre2[:], re2[:], AF.Abs)
                rlg2 = W("w1", N)
                v.tensor_tensor(rlg2[:], absre2[:], r2v[:], OP.add)
                glg2 = W("w5", N)
                v.tensor_tensor(glg2[:], G2[:], rlg2[:], OP.mult)
                SSv = W("w3", N)                 # -> becomes M2 after swap
                sc.activation(SSv[:], glg2[:], AF.Sqrt)
                rcl2 = W("w6", N)
                v.reciprocal_approx_fast(rcl2[:], rlg2[:])
                t12 = W("w1", N)
                v.tensor_tensor(t12[:], sqim2[:], rcl2[:], OP.mult)
                gsm2 = W("w4", N)
                v.tensor_tensor(gsm2[:], G2[:], t12[:], OP.mult)
                TTv = W("w5", N)                 # -> becomes P2 after swap
                sc.activation(TTv[:], gsm2[:], AF.Sqrt)
                TTv2 = W("w6", N)
                sc.activation(TTv2[:], gsm2[:], AF.Sqrt)
                m2 = W("w1", N)
                v.tensor_scalar(m2[:], re2[:], 0.0, None, OP.is_ge)
                # P2 = m2 ? SSv : TTv ; M2 = m2 ? TTv : SSv
                v.copy_predicated(TTv[:], m2[:].bitcast(I32), SSv[:])
                v.copy_predicated(SSv[:], m2[:].bitcast(I32), TTv2[:])
                P2 = TTv
                M2 = SSv

                zd = W("w0", N)
                v.tensor_scalar(zd[:], U2V[:], zs2c[:, t : t + 1], EPS, OP.mult, OP.add)
                sqzd = W("w2", N)
                sc.activation(sqzd[:], zd[:], AF.Square)
                ndn = W("w4", N)
                v.tensor_scalar(ndn[:], sqzd[:], EPS2, None, OP.add)
                rnd = W("w2", N)
                v.reciprocal_approx_fast(rnd[:], ndn[:])
                # endpoint trapezoid weights ride on rnd (shared by re & im)
                v.tensor_scalar(rnd[:, 0:1], rnd[:, 0:1], 0.5, None, OP.mult)
                v.tensor_scalar(rnd[:, N - 1 : N], rnd[:, N - 1 : N], 0.5, None, OP.mult)

                P2m = W("w6", N)
                v.tensor_scalar(P2m[:], P2[:], -1.0, None, OP.add)
                A12 = W("w1", N)
                v.tensor_tensor(A12[:], P2m[:], zd[:], OP.mult)
                A4 = W("w4", N)
                v.scalar_tensor_tensor(A4[:], M2[:], -EPS, A12[:], OP.mult, OP.add)
                igre = W("w1", N)
                v.scalar_tensor_tensor(
                    igre[:], A4[:], 1.0, rnd[:], OP.mult, OP.mult,
                    accum_out=accVre[:, t : t + 1],
                )
                B1 = W("w5", N)
                v.tensor_tensor(B1[:], M2[:], zd[:], OP.mult)
                B3 = W("w0", N)
                v.scalar_tensor_tensor(B3[:], P2m[:], EPS, B1[:], OP.mult, OP.add)
                igim = W("w3", N)
                v.scalar_tensor_tensor(
                    igim[:], B3[:], 1.0, rnd[:], OP.mult, OP.mult,
                    accum_out=accVim[:, t : t + 1],
                )

            # ---------------- finals ----------------
            Lre_f = cst.tile([128, NT], F32)
            v.tensor_tensor(Lre_f[:], accLre[:], sL[:], OP.mult)
            Lim_f = cst.tile([128, NT], F32)
            v.tensor_tensor(Lim_f[:], accLim[:], sLn[:], OP.mult)
            Vraw = cst.tile([128, NT], F32)
            v.tensor_tensor(Vraw[:], accVre[:], sV[:], OP.mult)
            Vre_f = cst.tile([128, NT], F32)
            v.scalar_tensor_tensor(Vre_f[:], invz[:], -2.0 * math.pi, Vraw[:], OP.mult, OP.add)
            Vim_f = cst.tile([128, NT], F32)
            v.tensor_tensor(Vim_f[:], accVim[:], sVn[:], OP.mult)

            for row, tile in ((0, Lre_f), (1, Lim_f), (2, Vre_f), (3, Vim_f)):
                nc.sync.dma_start(
                    out=out_d[row, :].rearrange("(t p) -> p t", p=128), in_=tile[:]
                )
    return nc


_NC_CACHE = {}


def kernel(a, b, zs):
    a = np.asarray(a, dtype=np.float32)
    b = np.asarray(b, dtype=np.float32)
    zs = np.asarray(zs, dtype=np.float32)
    if "nc" not in _NC_CACHE:
        nc0 = build_nc()
        nc0.finalize()
        _NC_CACHE["nc"] = nc0
    nc = _NC_CACHE["nc"]
    in_maps = [
        {"a": a, "b": b, "zs": zs[i * BLOC : (i + 1) * BLOC].copy()}
        for i in range(NCORES)
    ]
    res = bass_utils.run_bass_kernel_spmd(nc, in_maps, core_ids=list(range(NCORES)))
    out = np.concatenate([res.results[i]["out"] for i in range(NCORES)], axis=1)
    return out.astype(np.float32)


if __name__ == "__main__":
    rng = np.random.default_rng(0)
    out = kernel(
        rng.standard_normal(5).astype(np.float32),
        rng.standard_normal(5).astype(np.float32),
        (0.02 + 0.975 * rng.random(8192)).astype(np.float32),
    )
    print(out.shape, out.dtype, out[:, :3])
